# revision 1
# baseline (speedup 1.0000x reference)
"""Gated-relative-position-bias multi-head attention, 8-way tensor-parallel
over heads on Trainium2 (Bass/Tile).

Shapes: x (2, 2048, 1024), 16 heads x 64 head-dim, position_bias
(16, 2048, 2048), per-query sigmoid gates computed from x.

Sharding: core c owns heads (2c, 2c+1) = feature slice [128c, 128c+128).
Each core computes q/k/v for its heads, the gated-bias attention, and a
partial output projection (O_g @ Wo_g.T).  The host sums the 8 partials and
adds bo.

Per-core dataflow (v4 - PE-continuity, all-bf16 compute):
  - qT/kT/vT bf16 computed weights-stationary from xT chunks (1/sqrt(hd)
    folded into Wq/bq on the host).
  - scores are TRANSPOSED: sT[k, q] = kT.T @ qT (K=hd=64), so the attn @ v
    contraction (over k) has k on partitions.
  - gated position bias enters the scores PSUM via an identity matmul
    (psum += I.T @ pbg); pbg = pbT_chunk * gate_bcast computed two
    iterations ahead on DVE/GpSimd (bf16 2x on DVE).
  - loop order: h -> q-block(1024) -> kc(128 keys) -> b, so each pb chunk is
    DMA'd once and reused for both batches.
  - exp needs no max-subtraction (scores are O(+-3)); ACT evacuates each
    [128, 1024] scores PSUM pair with one wide bf16 exp.
  - softmax denominators come free as the ones-column (row 64) of va;
    normalization: reciprocal on DVE (input staged to a partition-0 tile -
    reciprocal_approx_fast misreads partition offsets), partition-broadcast
    on GpSimd, multiply on DVE.
  - output projection for a finished q-block runs inside the h=1 passes so
    its PSUM evacuation (DVE/ACT alternating) + bf16 partial DMA overlap
    the remaining attention.
"""

import sys

sys.path.insert(0, "/opt/trn_rl_repo")

import ml_dtypes
import numpy as np

import concourse.mybir as mybir
import concourse.tile as tile
from concourse import bacc
from concourse.bass_utils import run_bass_kernel_spmd

F32 = mybir.dt.float32
BF16 = mybir.dt.bfloat16
AF = mybir.ActivationFunctionType
ALU = mybir.AluOpType

B, T, D, H, HD = 2, 2048, 1024, 16, 64
NCORES = 8
HPC = H // NCORES          # heads per core = 2
FPC = HPC * HD             # features per core = 128
BT = B * T                 # 4096
P = 128
NKC = T // P               # key chunks = 16
QB = 512                   # q-block width
NQB = T // QB              # q-blocks per batch = 4

# test.py hooks
TRACE = False
LAST_RESULT = None


def _build_program():
    nc = bacc.Bacc("TRN2", target_bir_lowering=False, debug=False,
                   num_devices=NCORES)

    xT = nc.dram_tensor("xT", [D, BT], BF16, kind="ExternalInput")
    xg = nc.dram_tensor("xg", [P, BT], BF16, kind="ExternalInput")
    wq = nc.dram_tensor("wq", [D, FPC], BF16, kind="ExternalInput")
    wk = nc.dram_tensor("wk", [D, FPC], BF16, kind="ExternalInput")
    wv = nc.dram_tensor("wv", [D, FPC], BF16, kind="ExternalInput")
    bq = nc.dram_tensor("bq", [FPC], F32, kind="ExternalInput")
    bk = nc.dram_tensor("bk", [FPC], F32, kind="ExternalInput")
    bv = nc.dram_tensor("bv", [FPC], F32, kind="ExternalInput")
    wo = nc.dram_tensor("wo", [FPC, D], BF16, kind="ExternalInput")
    pbt = nc.dram_tensor("pbt", [HPC, T, T], BF16, kind="ExternalInput")
    wg2 = nc.dram_tensor("wg2", [P, 97], BF16, kind="ExternalInput")
    bg2 = nc.dram_tensor("bg2", [97], F32, kind="ExternalInput")
    gc2 = nc.dram_tensor("gc2", [97], F32, kind="ExternalInput")
    idb = nc.dram_tensor("idb", [P, P], BF16, kind="ExternalInput")
    idb8d = nc.dram_tensor("idb8", [P, P], mybir.dt.float8e4,
                           kind="ExternalInput")
    out = nc.dram_tensor("out", [BT, D], BF16, kind="ExternalOutput")

    with tile.TileContext(nc) as tc, \
         tc.tile_pool(name="const", bufs=1) as const, \
         tc.tile_pool(name="big", bufs=1) as big, \
         tc.tile_pool(name="xt", bufs=2) as xt_pool, \
         tc.tile_pool(name="xgp", bufs=2) as xg_pool, \
         tc.tile_pool(name="gtmp", bufs=1) as gtmp_pool, \
         tc.tile_pool(name="pb", bufs=9) as pb_pool, \
         tc.tile_pool(name="pbg", bufs=10) as pbg_pool, \
         tc.tile_pool(name="ex0", bufs=4) as ex_pool0, \
         tc.tile_pool(name="ex1", bufs=4) as ex_pool1, \
         tc.tile_pool(name="nrm", bufs=4) as nrm_pool, \
         tc.tile_pool(name="osb", bufs=4) as osb_pool, \
         tc.tile_pool(name="psA", bufs=2, space="PSUM") as psA, \
         tc.tile_pool(name="psS", bufs=4, space="PSUM") as psS, \
         tc.tile_pool(name="psO", bufs=2, space="PSUM") as psO:
        # ---------------- constants ----------------
        idb_t = const.tile([P, P], BF16, tag="idb")
        nc.sync.dma_start(idb_t[:], idb[:])
        idb8_t = const.tile([P, P], mybir.dt.float8e4, tag="idb8")
        nc.sync.dma_start(idb8_t[:], idb8d[:])
        w_ts = {}
        for name, dram in (("wq", wq), ("wk", wk), ("wv", wv)):
            w_t = const.tile([P, D // P, FPC], BF16, tag=name, name=name + "w")
            nc.sync.dma_start(w_t[:], dram.rearrange("(c p) f -> p c f", p=P))
            w_ts[name] = w_t
        b_ts = {}
        for name, dram in (("bq", bq), ("bk", bk), ("bv", bv)):
            b_t = const.tile([FPC, 1], F32, tag=name, name=name + "b")
            nc.sync.dma_start(b_t[:], dram.rearrange("(p o) -> p o", o=1))
            b_ts[name] = b_t
        wo_t = const.tile([FPC, D], BF16, tag="wo")
        nc.sync.dma_start(wo_t[:], wo[:])
        wg2_t = const.tile([P, 97], BF16, tag="wg2")
        nc.sync.dma_start(wg2_t[:], wg2[:])
        bg2_t = const.tile([97, 1], F32, tag="bg2")
        nc.sync.dma_start(bg2_t[:], bg2.rearrange("(p o) -> p o", o=1))
        gc_t = const.tile([97, 1], F32, tag="gc")
        nc.sync.dma_start(gc_t[:], gc2.rearrange("(p o) -> p o", o=1))

        qT = big.tile([FPC, BT], BF16, tag="qT")
        kT = big.tile([FPC, BT], BF16, tag="kT")
        vT = big.tile([FPC, BT], BF16, tag="vT")
        G2h = [big.tile([1, BT], BF16, tag=f"G2h{h}", name=f"G2h{h}")
               for h in range(HPC)]
        OT = [big.tile([FPC, T], BF16, tag=f"OT{b}", name=f"OT{b}")
              for b in range(B)]
        # va[(h,b)]: [keys=128, kc, 64 v-cols + ones col]
        va2 = {(h, b): big.tile([P, NKC, HD + 1], BF16,
                                tag=f"va{h}{b}", name=f"va{h}{b}")
               for h in range(HPC) for b in range(B)}
        G = big.tile([97, BT], BF16, tag="G")
        gbcs = {(h, b): big.tile([P, T], BF16, tag=f"gbc{h}{b}",
                                 name=f"gbc{h}{b}")
                for h in range(HPC) for b in range(B)}

        passes = [(h, qb) for qb in range(NQB) for h in range(HPC)]
        pb_store = {}
        pbg_store = {}

        def load_pb(pi, kp):
            ph, pqb = passes[pi]
            pb_t = pb_pool.tile([P, QB], BF16, tag="pb",
                                name=f"pb{pi}_{kp}")
            nc.sync.dma_start(
                pb_t[:], pbt[ph, kp * P:(kp + 1) * P,
                             pqb * QB:(pqb + 1) * QB])
            pb_store[(pi, kp)] = pb_t

        def make_pbg(pi, kp, bsel=None):
            # gated bias, computed four kc-iterations ahead of the PE (DVE)
            ph, pqb = passes[pi]
            for b in range(B) if bsel is None else [bsel]:
                pbg = pbg_pool.tile([P, QB], BF16, tag="pbg",
                                    name=f"pbg{b}")
                nc.vector.tensor_tensor(
                    out=pbg[:], in0=pb_store[(pi, kp)][:],
                    in1=gbcs[(ph, b)][:, pqb * QB:(pqb + 1) * QB],
                    op=ALU.mult)
                pbg_store[(pi, kp, b)] = pbg

        # ---------------- q/k/v projections + gate ----------------
        # setup work (gate combine, broadcasts, v transposes) is interleaved
        # into the projection loop so the attention phase starts immediately
        for h in range(HPC):
            for b in range(B):
                nc.vector.memset(va2[(h, b)][:], 1.0)
        gt1 = gtmp_pool.tile([33, BT], BF16, tag="gt1")
        for th in range(8):
            tw = BT // 8
            if th == 1:
                for kp in range(7):
                    load_pb(0, kp)
            tsl = slice(th * tw, (th + 1) * tw)
            xt_t = xt_pool.tile([P, D // P, tw], BF16, tag="xt",
                                name=f"xt{th}")
            nc.sync.dma_start(
                xt_t[:], xT.rearrange("(c p) t -> p c t", p=P)[:, :, tsl])
            for ti in range(tw // 512):
                c0 = th * tw + ti * 512
                for wname, bname, dst in (("wq", "bq", qT), ("wk", "bk", kT),
                                          ("wv", "bv", vT)):
                    psq = psA.tile([FPC, 512], F32, tag="av",
                                   name=f"psq{wname}{th}{ti}")
                    for d in range(D // P):
                        nc.tensor.matmul(
                            psq[:], w_ts[wname][:, d, :],
                            xt_t[:, d, ti * 512:(ti + 1) * 512],
                            start=(d == 0), stop=(d == D // P - 1))
                    nc.vector.tensor_scalar(
                        out=dst[:, c0:c0 + 512], in0=psq[:],
                        scalar1=b_ts[bname][:], scalar2=None, op0=ALU.add)
                xg_t = xg_pool.tile([P, 512], BF16, tag="xgc")
                nc.sync.dma_start(xg_t[:], xg[:, c0:c0 + 512])
                psg = psA.tile([97, 512], F32, tag="av", name=f"psg{th}{ti}")
                nc.tensor.matmul(psg[:], wg2_t[:], xg_t[:],
                                 start=True, stop=True)
                nc.scalar.activation(G[:, c0:c0 + 512], psg[:], AF.Sigmoid,
                                     bias=bg2_t[:])
                # gate combine for this token slice: G2 = a*(b*gc - 1) + 2
                for h in range(HPC):
                    r = 32 * h
                    csl = slice(c0, c0 + 512)
                    nc.vector.tensor_scalar(
                        out=gt1[r:r + 1, csl], in0=G[64 + r:65 + r, csl],
                        scalar1=gc_t[64 + r:65 + r, :], scalar2=-1.0,
                        op0=ALU.mult, op1=ALU.add)
                    nc.vector.tensor_mul(G2h[h][0:1, csl],
                                         G[r:r + 1, csl], gt1[r:r + 1, csl])
                    nc.vector.tensor_scalar(out=G2h[h][0:1, csl],
                                            in0=G2h[h][0:1, csl],
                                            scalar1=2.0, scalar2=None,
                                            op0=ALU.add)
            # transpose the freshly computed v slice into va (+ ones col)
            vb = th // 4
            for h in range(HPC):
                hsl = slice(h * HD, (h + 1) * HD)
                for kk in range(4):
                    kc = (th % 4) * 4 + kk
                    pst = psS.tile([P, HD], BF16, tag="sc",
                                   name=f"pst{th}{h}{kk}")
                    nc.tensor.transpose(
                        pst[:],
                        vT[hsl, vb * T + kc * P: vb * T + (kc + 1) * P],
                        idb_t[hsl, hsl])
                    nc.any.tensor_copy(va2[(h, vb)][:, kc, 0:HD], pst[:])
            # batch-b gates complete -> broadcast them (gpsimd)
            if th % 4 == 3:
                for h in range(HPC):
                    nc.gpsimd.partition_broadcast(
                        gbcs[(h, vb)][:],
                        G2h[h][0:1, vb * T:(vb + 1) * T])
                if vb == 0:
                    for kp in range(4):
                        make_pbg(0, kp, bsel=0)

        # ---------------- attention ----------------
        for kp in range(4):
            make_pbg(0, kp, bsel=1)
        for pi, (h, qb) in enumerate(passes):
            hsl = slice(h * HD, (h + 1) * HD)
            q0 = qb * QB          # within-batch q offset
            avs = {b: psA.tile([HD + 1, QB], F32, tag="av",
                               name=f"av{h}{qb}{b}")
                   for b in range(B)}
            # deferred attn @ v matmuls: exp(kc) is consumed during
            # iteration kc+2 so the PE never waits on the ACT engine
            pend_av = {b: [] for b in range(B)}

            def flush_av(b, depth, avs=avs, pend_av=pend_av):
                while len(pend_av[b]) > depth:
                    kcp, ex = pend_av[b].pop(0)
                    nc.tensor.matmul(
                        avs[b][:], va2[(avs_h, b)][:, kcp, :], ex[:],
                        start=(kcp == 0), stop=(kcp == NKC - 1))

            avs_h = h
            for kc in range(NKC):
                tgt = kc + 7
                if tgt < NKC:
                    load_pb(pi, tgt)
                elif pi + 1 < len(passes):
                    load_pb(pi + 1, tgt - NKC)
                tgt = kc + 4
                if tgt < NKC:
                    make_pbg(pi, tgt)
                elif pi + 1 < len(passes):
                    make_pbg(pi + 1, tgt - NKC)
                pb_store.pop((pi, kc - 1), None)
                for b in range(B):
                    flush_av(b, 1)
                    pbg = pbg_store.pop((pi, kc, b))
                    sc = psS.tile([P, QB], F32, tag="sc",
                                  name=f"sc{h}{qb}{kc}{b}")
                    lk = kT[hsl, b * T + kc * P: b * T + (kc + 1) * P]
                    # scores as a complete (fast-class) group; the bias
                    # inject reopens and accumulates (stop is a hw no-op)
                    nc.tensor.matmul(
                        sc[:], lk,
                        qT[hsl, b * T + q0: b * T + q0 + QB],
                        start=True, stop=True)
                    nc.tensor.matmul(sc[:], idb8_t[:], pbg[:],
                                     start=False, stop=True,
                                     skip_group_check=True)
                    pool = ex_pool0 if b == 0 else ex_pool1
                    ex = pool.tile([P, QB], BF16, tag="ex",
                                   name=f"ex{b}")
                    nc.scalar.activation(ex[:], sc[:], AF.Exp)
                    pend_av[b].append((kc, ex))
            for b in range(B):
                flush_av(b, 0)
            # normalize: softmax denominators live in row HD of avs
            for b in range(B):
                av = avs[b]
                dn = nrm_pool.tile([1, QB], F32, tag="dn")
                nc.vector.tensor_copy(dn[:], av[HD:HD + 1, :])
                rc = nrm_pool.tile([1, QB], F32, tag="rc")
                nc.vector.reciprocal_approx_fast(rc[:], dn[:])
                rb = nrm_pool.tile([HD, QB], F32, tag="rb")
                nc.gpsimd.partition_broadcast(rb[:], rc[:])
                nc.vector.tensor_tensor(
                    out=OT[b][hsl, q0: q0 + QB],
                    in0=av[0:HD, :], in1=rb[:], op=ALU.mult)
            # output projection once both head rows of OT are done
            if h == HPC - 1:
                for b in range(B):
                    for tt in range(QB // P):
                        t0 = q0 + tt * P
                        ob = osb_pool.tile([P, 2, 512], BF16, tag="ob")
                        for s in range(D // 512):
                            po = psO.tile([P, 512], F32, tag="po",
                                          name=f"po{qb}{b}{tt}{s}")
                            nc.tensor.matmul(
                                po[:], OT[b][:, t0:t0 + P],
                                wo_t[:, s * 512:(s + 1) * 512],
                                start=True, stop=True)
                            if s % 2 == 1:
                                nc.scalar.activation(ob[:, s, :], po[:],
                                                     AF.Copy)
                            else:
                                nc.vector.tensor_copy(ob[:, s, :], po[:])
                        nc.sync.dma_start(
                            out[b * T + t0: b * T + t0 + P, :], ob[:])

    nc.compile()
    return nc


_PROGRAM = None


def _get_program():
    global _PROGRAM
    if _PROGRAM is None:
        _PROGRAM = _build_program()
    return _PROGRAM


def kernel(x, position_bias, Wq, bq, Wk, bk, Wv, bv, Wo, bo, Wg, bg,
           gru_const):
    global LAST_RESULT
    x = np.asarray(x, dtype=np.float32)
    position_bias = np.asarray(position_bias, dtype=np.float32)
    Wq = np.asarray(Wq, dtype=np.float32)
    Wk = np.asarray(Wk, dtype=np.float32)
    Wv = np.asarray(Wv, dtype=np.float32)
    Wo = np.asarray(Wo, dtype=np.float32)
    bq = np.asarray(bq, dtype=np.float32)
    bk = np.asarray(bk, dtype=np.float32)
    bv = np.asarray(bv, dtype=np.float32)
    bo = np.asarray(bo, dtype=np.float32)
    Wg = np.asarray(Wg, dtype=np.float32)
    bg = np.asarray(bg, dtype=np.float32)
    gru_const = np.asarray(gru_const, dtype=np.float32)

    scale = np.float32(1.0 / np.sqrt(np.float32(HD)))

    xT_np = np.ascontiguousarray(x.reshape(BT, D).T)           # [D, BT]
    idb_np = np.eye(P).astype(ml_dtypes.bfloat16)
    # the reshape-(2,4)-sum of the 8 gate features is linear -> fold into
    # the weights:  Wg2[g] = sum of Wg rows [4g, 4g+4)
    Wg2 = Wg.reshape(2, 4, HD).sum(1)                          # [2, HD]
    bg2v = bg.reshape(2, 4).sum(1)                             # [2]

    in_maps = []
    for c in range(NCORES):
        fsl = slice(c * FPC, (c + 1) * FPC)
        wg2_np = np.zeros((P, 97), dtype=np.float32)
        bg2_np = np.zeros((97,), dtype=np.float32)
        # rows 0/32 = gate-a for head0/head1; rows 64/96 = gate-b
        wg2_np[0:HD, 0] = Wg2[0]
        wg2_np[HD:P, 32] = Wg2[0]
        wg2_np[0:HD, 64] = Wg2[1]
        wg2_np[HD:P, 96] = Wg2[1]
        bg2_np[[0, 32]] = bg2v[0]
        bg2_np[[64, 96]] = bg2v[1]
        gc2_np = np.zeros((97,), dtype=np.float32)
        gc2_np[64] = gru_const[0, c * HPC, 0, 0]
        gc2_np[96] = gru_const[0, c * HPC + 1, 0, 0]
        in_maps.append({
            "xT": xT_np.astype(ml_dtypes.bfloat16),
            "xg": np.ascontiguousarray(xT_np[fsl, :]).astype(ml_dtypes.bfloat16),
            "wq": (np.ascontiguousarray(Wq.T[:, fsl]) * scale).astype(ml_dtypes.bfloat16),
            "wk": np.ascontiguousarray(Wk.T[:, fsl]).astype(ml_dtypes.bfloat16),
            "wv": np.ascontiguousarray(Wv.T[:, fsl]).astype(ml_dtypes.bfloat16),
            "bq": np.ascontiguousarray(bq[fsl]) * scale,
            "bk": np.ascontiguousarray(bk[fsl]),
            "bv": np.ascontiguousarray(bv[fsl]),
            "wo": np.ascontiguousarray(Wo[:, fsl].T).astype(ml_dtypes.bfloat16),
            "pbt": np.ascontiguousarray(
                position_bias[c * HPC:(c + 1) * HPC].transpose(0, 2, 1)
            ).astype(ml_dtypes.bfloat16),
            "wg2": wg2_np.astype(ml_dtypes.bfloat16),
            "bg2": bg2_np,
            "gc2": gc2_np,
            "idb": idb_np,
            "idb8": np.eye(P).astype(ml_dtypes.float8_e4m3fn),
        })

    nc = _get_program()
    res = run_bass_kernel_spmd(nc, in_maps, core_ids=list(range(NCORES)),
                               trace=TRACE)
    LAST_RESULT = res
    acc = res.results[0]["out"].astype(np.float32).copy()
    for c in range(1, NCORES):
        acc += res.results[c]["out"].astype(np.float32)
    acc += bo[None, :]
    return acc.reshape(B, T, D)

